# revision 6
# baseline (speedup 1.0000x reference)
"""GQA attention (B=2, L=2048, D=2048, H=16, KVH=4, Dh=128, RoPE, causal)
distributed over 8 TRN2 NeuronCores.

Sharding: DP on batch (2) x TP on head-groups (4). Core c handles batch
b=c//4, head group g=c%4 (q heads 4g..4g+3, kv head g). Per lq-block the
attention outputs O_g^T (4 heads, [512, LB], transposed) are AllGather'd
in ONE collective across the 4 TP cores; each core then computes a
distinct 512-column slice of the output projection (Wo column-sharded),
so the big [L, D] activation never rides a collective.

v2 restructure vs v1:
- per-block software pipeline proj(b) -> attn(b) -> AG(b) -> outproj(b)
  so the first AllGather issues at ~25us (was 172us) and comm latency
  hides under later blocks' projection/attention compute.
- 4 batched AllGathers (one per block) instead of 16 per-(block,head)
  calls: same bytes, 1/4 the per-call barrier latency.
- softmax denominator: all-ones [128,128] matmul broadcasts column sums
  to every partition in one PE op; 1/x via the single-instruction DVE
  reciprocal_approx_fast on [128,512] (v1: [1,512] bit-exact reciprocal
  at 3.3us + 1us GpSimd PartitionBroadcast, both on the critical path).
- diagonal S/PV tiles compute only the live [c0:] column range (no
  memsets, narrower matmuls).
- DMA order: x block 0 first, then qkv weights k-ascending, wo last, so
  the first projection matmul starts ~8us into the kernel.
"""
import sys
sys.path.insert(0, '/opt/trn_rl_repo')
import math
import numpy as np

B, L, D = 2, 2048, 2048
H, KVH, Dh = 16, 4, 128
HPC = H // KVH          # q heads per core = 4
NG = 4                  # TP group size
P = 128                 # partitions
LB = 512                # lq block size
NLB = L // LB           # 4 lq blocks
NKT = D // P            # 16 k-tiles over D
NLT = L // P            # 16 tiles over L
HD = HPC * Dh           # 512
SCALE = 1.0 / math.sqrt(Dh)
NEG = -1.0e30

_cache = {}
COMPUTE = "bf16"  # "f32r" | "bf16" | "f32"


def _build():
    from concourse import bacc, bass, mybir, tile

    F32 = mybir.dt.float32
    F32R = mybir.dt.float32r
    BF16 = mybir.dt.bfloat16
    SB = {"f32r": F32R, "bf16": BF16, "f32": F32}[COMPUTE]
    AF = mybir.ActivationFunctionType
    MULT = mybir.AluOpType.mult
    ADD = mybir.AluOpType.add

    nc = bacc.Bacc(None, target_bir_lowering=False, num_devices=8)

    xT = nc.declare_dram_parameter("xT", [D, L], SB, isOutput=False)
    cosT = nc.declare_dram_parameter("cosT", [Dh, L], SB, isOutput=False)
    sinT = nc.declare_dram_parameter("sinT", [Dh, L], SB, isOutput=False)
    wq = nc.declare_dram_parameter("wq", [D, HD], SB, isOutput=False)
    wk = nc.declare_dram_parameter("wk", [D, Dh], SB, isOutput=False)
    wv = nc.declare_dram_parameter("wv", [D, Dh], SB, isOutput=False)
    wo = nc.declare_dram_parameter("wo", [D, LB], SB, isOutput=False)
    rot = nc.declare_dram_parameter("rot", [Dh, Dh], SB, isOutput=False)
    iden = nc.declare_dram_parameter("iden", [P, P], F32, isOutput=False)
    ones = nc.declare_dram_parameter("ones", [P, P], SB, isOutput=False)
    masks = nc.declare_dram_parameter("masks", [P, P], SB, isOutput=False)
    outT = nc.declare_dram_parameter("outT", [LB, L], F32, isOutput=True)

    def mmul(out, lhsT, rhs, **kw):
        nc.tensor.matmul(out, lhsT, rhs, **kw)

    with tile.TileContext(nc) as tc, \
         nc.allow_low_precision(reason="softmax in bf16, approx reciprocal"):
        with tc.tile_pool(name="dram", bufs=1, space="DRAM") as dram, \
             tc.tile_pool(name="const", bufs=1) as cpool, \
             tc.tile_pool(name="acts", bufs=1) as apool, \
             tc.tile_pool(name="wpool", bufs=1) as wpool, \
             tc.tile_pool(name="xpool", bufs=1) as xpool, \
             tc.tile_pool(name="work", bufs=1) as tpool, \
             tc.tile_pool(name="psum", bufs=1, space="PSUM") as psum:

            ag_in = [dram.tile([NG * P, LB], SB, name=f"agin{j}")
                     for j in range(NLB)]
            ag_out = [dram.tile([NG * NG * P, LB], SB, name=f"agout{j}")
                      for j in range(NLB)]

            # ---------------- constants ----------------
            rot_t = cpool.tile([Dh, Dh], SB)
            iden_t = cpool.tile([P, P], F32)
            ones_t = cpool.tile([P, P], SB)
            mask_t = cpool.tile([P, P], SB)
            cos_t = cpool.tile([Dh, L], SB)
            sin_t = cpool.tile([Dh, L], SB)
            nc.sync.dma_start(rot_t[:], rot[:])
            nc.sync.dma_start(iden_t[:], iden[:])
            nc.sync.dma_start(ones_t[:], ones[:])
            nc.sync.dma_start(mask_t[:], masks[:])
            nc.sync.dma_start(cos_t[:], cosT[:])
            nc.sync.dma_start(sin_t[:], sinT[:])

            # persistent activations
            qkT = apool.tile([Dh, HPC + 1, L], SB)  # 4 q heads + k
            vnat = apool.tile([P, NLT, Dh], SB)     # v natural layout

            # weights (DMA'd up front; wo deferred)
            wq_t = wpool.tile([P, NKT, HD], SB)
            wk_t = wpool.tile([P, NKT, Dh], SB)
            wv_t = wpool.tile([P, NKT, Dh], SB)
            wo_t = wpool.tile([P, NKT, LB], SB)

            xt = {}

            def xt_load(blk, ck=4):
                lqs = slice(blk * LB, (blk + 1) * LB)
                t = xpool.tile([P, NKT, LB], SB, name=f"xt{blk}",
                               tag="xt3", bufs=4)
                for k0 in range(0, NKT, ck):
                    nc.sync.dma_start(
                        t[:, k0:k0 + ck, :],
                        xT[k0 * P:(k0 + ck) * P, lqs].rearrange(
                            "(j p) n -> p j n", p=P))
                xt[blk] = t

            gt = {}

            def gt_load(blk):
                t = xpool.tile([P, NKT, LB], SB, name=f"gt{blk}",
                               tag="gt3", bufs=2)
                for j in range(NKT):
                    nc.gpsimd.dma_start(
                        t[:, j, :], ag_out[blk][j * P:(j + 1) * P, :])
                gt[blk] = t

            # x block 0 + wq first (fine chunks, parallel queues), then wk/wv
            xt_load(0, ck=2)
            for k0 in range(0, NKT, 2):
                nc.sync.dma_start(
                    wq_t[:, k0:k0 + 2, :],
                    wq[k0 * P:(k0 + 2) * P, :].rearrange(
                        "(j p) n -> p j n", p=P))
            for k0 in range(0, NKT, 4):
                nc.sync.dma_start(
                    wk_t[:, k0:k0 + 4, :],
                    wk[k0 * P:(k0 + 4) * P, :].rearrange(
                        "(j p) n -> p j n", p=P))
                nc.sync.dma_start(
                    wv_t[:, k0:k0 + 4, :],
                    wv[k0 * P:(k0 + 4) * P, :].rearrange(
                        "(j p) n -> p j n", p=P))
            for b in range(1, NLB):
                xt_load(b)

            def wo_load():
                for k0 in range(0, NKT, 4):
                    nc.sync.dma_start(
                        wo_t[:, k0:k0 + 4, :],
                        wo[k0 * P:(k0 + 4) * P, :].rearrange(
                            "(j p) n -> p j n", p=P))

            # ---------------- phases ----------------
            def proj(blk):
                lqs = slice(blk * LB, (blk + 1) * LB)
                x3 = xt[blk]
                vts = xpool.tile([Dh, LB], F32, name=f"vts{blk}",
                                 tag="vts", bufs=2)
                for m in range(HPC + 2):
                    pj = psum.tile([P, LB], F32, name=f"pj{blk}_{m}",
                                   tag="pj", bufs=2)
                    for kk in range(NKT):
                        if m < HPC:
                            wsl = wq_t[:, kk, m * Dh:(m + 1) * Dh]
                        elif m == HPC:
                            wsl = wk_t[:, kk, :]
                        else:
                            wsl = wv_t[:, kk, :]
                        mmul(pj[:], wsl, x3[:, kk, :],
                             start=(kk == 0), stop=(kk == NKT - 1))
                    if m == HPC + 1:
                        # v: stage vT, transpose to natural layout
                        nc.vector.tensor_copy(vts[:], pj[:])
                        for j in range(4):
                            vp = psum.tile([P, LB], F32, name=f"vp{blk}_{j}",
                                           tag="sq", bufs=2)
                            nc.tensor.transpose(
                                vp[:, :P], vts[:, j * P:(j + 1) * P],
                                iden_t[:])
                            nc.vector.tensor_copy(
                                vnat[:, blk * 4 + j, :], vp[:, :P])
                    else:
                        # RoPE: q' = cos*q + sin*(R q)
                        qs = tpool.tile([Dh, LB], SB, name=f"qs{blk}_{m}",
                                        tag="qs", bufs=2)
                        nc.vector.tensor_copy(qs[:], pj[:])
                        rq = psum.tile([Dh, LB], F32, name=f"rq{blk}_{m}",
                                       tag="sq", bufs=2)
                        mmul(rq[:], rot_t[:], qs[:], start=True, stop=True)
                        t1 = tpool.tile([Dh, LB], SB, name=f"t1_{blk}_{m}",
                                        tag="t1", bufs=2)
                        nc.vector.tensor_tensor(
                            out=t1[:], in0=qs[:], in1=cos_t[:, lqs], op=MULT)
                        t2 = tpool.tile([Dh, LB], SB, name=f"t2_{blk}_{m}",
                                        tag="t2", bufs=2)
                        nc.vector.tensor_tensor(
                            out=t2[:], in0=rq[:], in1=sin_t[:, lqs], op=MULT)
                        nc.vector.tensor_tensor(
                            out=qkT[:, m, lqs], in0=t1[:], in1=t2[:], op=ADD)

            def attn(blk):
                nlk = (blk + 1) * NG  # causal lk tiles
                for h in range(HPC):
                    ot = psum.tile([Dh, LB], F32, name=f"ot{blk}_{h}",
                                   tag="ot", bufs=2)
                    racc = tpool.tile([P, LB], SB, name=f"racc{blk}_{h}",
                                      tag="racc", bufs=2)
                    for i in range(nlk):
                        di = i - NG * blk
                        c0 = di * P if di > 0 else 0
                        st = psum.tile([P, LB], F32, name=f"st{blk}_{h}_{i}",
                                       tag="sq", bufs=2)
                        mmul(st[:, c0:],
                             qkT[:, HPC, i * P:(i + 1) * P],
                             qkT[:, h, blk * LB + c0:(blk + 1) * LB],
                             start=True, stop=True)
                        if di >= 0:
                            nc.vector.tensor_tensor(
                                out=st[:, c0:c0 + P], in0=st[:, c0:c0 + P],
                                in1=mask_t[:], op=ADD)
                        pt = tpool.tile([P, LB], SB, name=f"pt{blk}_{h}_{i}",
                                        tag="pt", bufs=3)
                        nc.scalar.activation(pt[:, c0:], st[:, c0:],
                                             AF.Exp, scale=SCALE)
                        if i == 0:
                            mmul(ot[:], vnat[:, i, :], pt[:],
                                 start=True, stop=(i == nlk - 1))
                            nc.vector.tensor_copy(racc[:], pt[:])
                        else:
                            mmul(ot[:, c0:], vnat[:, i, :], pt[:, c0:],
                                 start=False, stop=(i == nlk - 1))
                            nc.vector.tensor_tensor(
                                out=racc[:, c0:], in0=racc[:, c0:],
                                in1=pt[:, c0:], op=ADD)
                    # denominator: ones-matmul broadcasts column sums to all
                    # 128 partitions; approx reciprocal is 1 DVE instruction
                    rsb = psum.tile([P, LB], F32, name=f"rsb{blk}_{h}",
                                    tag="sq", bufs=2)
                    mmul(rsb[:], ones_t[:], racc[:], start=True, stop=True)
                    rb = tpool.tile([P, LB], F32, name=f"rb{blk}_{h}",
                                    tag="rb", bufs=2)
                    nc.vector.reciprocal_approx_fast(rb[:], rsb[:])
                    od = tpool.tile([Dh, LB], SB, name=f"od{blk}_{h}",
                                    tag="od", bufs=2)
                    nc.vector.tensor_tensor(
                        out=od[:], in0=ot[:], in1=rb[:], op=MULT)
                    nc.scalar.dma_start(
                        ag_in[blk][h * P:(h + 1) * P, :], od[:])
                nc.gpsimd.collective_compute(
                    "AllGather",
                    mybir.AluOpType.bypass,
                    replica_groups=[[0, 1, 2, 3], [4, 5, 6, 7]],
                    ins=[ag_in[blk][:, :].opt()],
                    outs=[ag_out[blk][:, :].opt()],
                )

            def outproj(blk):
                lqs = slice(blk * LB, (blk + 1) * LB)
                g3 = gt[blk]
                for nt in range(4):
                    fp = psum.tile([P, LB], F32, name=f"fp{blk}_{nt}",
                                   tag="fp", bufs=2)
                    for kk in range(NKT):
                        mmul(fp[:], wo_t[:, kk, nt * P:(nt + 1) * P],
                             g3[:, kk, :],
                             start=(kk == 0), stop=(kk == NKT - 1))
                    ft = tpool.tile([P, LB], F32, name=f"ft{blk}_{nt}",
                                    tag="ft", bufs=2)
                    nc.vector.tensor_copy(ft[:], fp[:])
                    nc.scalar.dma_start(outT[nt * P:(nt + 1) * P, lqs],
                                        ft[:])

            # ---------------- pipeline ----------------
            for blk in range(NLB):
                proj(blk)
                attn(blk)
                gt_load(blk)
                if blk == 0:
                    wo_load()
            for blk in range(NLB):
                outproj(blk)
    return nc


def _np_dt():
    if COMPUTE == "bf16":
        import ml_dtypes
        return ml_dtypes.bfloat16
    return np.float32


def _prep(hidden_states, cos, sin, Wq, Wk, Wv, Wo):
    rot = np.zeros((Dh, Dh), dtype=np.float32)
    for p in range(Dh // 2):
        rot[p, p + Dh // 2] = 1.0
        rot[p + Dh // 2, p] = -1.0
    iden = np.eye(P, dtype=np.float32)
    ones = np.ones((P, P), dtype=np.float32)
    # triangular tile mask: masked where kl > qq (S^T diagonal tile)
    kl = np.arange(P)[:, None]
    qq = np.arange(P)[None, :]
    masks = np.where(kl > qq, NEG, 0.0).astype(np.float32)

    cosT = np.ascontiguousarray(cos.T)
    sinT = np.ascontiguousarray(sin.T)
    ndt = _np_dt()
    maps = []
    for c in range(8):
        b, g = divmod(c, NG)
        maps.append({
            "xT": np.ascontiguousarray(hidden_states[b].T).astype(ndt),
            "cosT": cosT.astype(ndt), "sinT": sinT.astype(ndt),
            "wq": np.ascontiguousarray(Wq[:, g * HD:(g + 1) * HD]).astype(ndt),
            "wk": np.ascontiguousarray(Wk[:, g * Dh:(g + 1) * Dh]).astype(ndt),
            "wv": np.ascontiguousarray(Wv[:, g * Dh:(g + 1) * Dh]).astype(ndt),
            "wo": np.ascontiguousarray(Wo[:, g * LB:(g + 1) * LB]).astype(ndt),
            "rot": rot.astype(ndt), "iden": iden,
            "ones": ones.astype(ndt), "masks": masks.astype(ndt),
        })
    return maps


def kernel(hidden_states, cos, sin, Wq, Wk, Wv, Wo):
    from concourse.bass_utils import run_bass_kernel_spmd

    hidden_states = np.asarray(hidden_states, dtype=np.float32)
    cos = np.asarray(cos, dtype=np.float32)
    sin = np.asarray(sin, dtype=np.float32)
    Wq = np.asarray(Wq, dtype=np.float32)
    Wk = np.asarray(Wk, dtype=np.float32)
    Wv = np.asarray(Wv, dtype=np.float32)
    Wo = np.asarray(Wo, dtype=np.float32)

    if "nc" not in _cache:
        nc = _build()
        nc.finalize()
        _cache["nc"] = nc
    nc = _cache["nc"]
    in_maps = _prep(hidden_states, cos, sin, Wq, Wk, Wv, Wo)
    res = run_bass_kernel_spmd(nc, in_maps, list(range(8)))
    _cache["last_result"] = res
    out = np.empty((B, L, D), dtype=np.float32)
    for c in range(8):
        b, g = divmod(c, NG)
        out[b, :, g * LB:(g + 1) * LB] = res.results[c]["outT"].T
    return out


# revision 7
# speedup vs baseline: 1.1051x; 1.1051x over previous
"""GQA attention (B=2, L=2048, D=2048, H=16, KVH=4, Dh=128, RoPE, causal)
distributed over 8 TRN2 NeuronCores.

Sharding: DP on batch (2) x TP on head-groups (4). Core c handles batch
b=c//4, head group g=c%4 (q heads 4g..4g+3, kv head g). Per lq-block the
attention outputs O_g^T (4 heads, [512, LB], transposed) are AllGather'd
in ONE collective across the 4 TP cores; each core then computes a
distinct 512-column slice of the output projection (Wo column-sharded),
so the big [L, D] activation never rides a collective.

v2 restructure vs v1:
- per-block software pipeline proj(b) -> attn(b) -> AG(b) -> outproj(b)
  so the first AllGather issues at ~25us (was 172us) and comm latency
  hides under later blocks' projection/attention compute.
- 4 batched AllGathers (one per block) instead of 16 per-(block,head)
  calls: same bytes, 1/4 the per-call barrier latency.
- softmax denominator: all-ones [128,128] matmul broadcasts column sums
  to every partition in one PE op; 1/x via the single-instruction DVE
  reciprocal_approx_fast on [128,512] (v1: [1,512] bit-exact reciprocal
  at 3.3us + 1us GpSimd PartitionBroadcast, both on the critical path).
- diagonal S/PV tiles compute only the live [c0:] column range (no
  memsets, narrower matmuls).
- DMA order: x block 0 first, then qkv weights k-ascending, wo last, so
  the first projection matmul starts ~8us into the kernel.
"""
import sys
sys.path.insert(0, '/opt/trn_rl_repo')
import math
import numpy as np

B, L, D = 2, 2048, 2048
H, KVH, Dh = 16, 4, 128
HPC = H // KVH          # q heads per core = 4
NG = 4                  # TP group size
P = 128                 # partitions
LB = 512                # lq block size
NLB = L // LB           # 4 lq blocks
NKT = D // P            # 16 k-tiles over D
NLT = L // P            # 16 tiles over L
HD = HPC * Dh           # 512
SCALE = 1.0 / math.sqrt(Dh)
NEG = -1.0e30

_cache = {}
COMPUTE = "bf16"  # "f32r" | "bf16" | "f32"


def _build():
    from concourse import bacc, bass, mybir, tile

    F32 = mybir.dt.float32
    F32R = mybir.dt.float32r
    BF16 = mybir.dt.bfloat16
    SB = {"f32r": F32R, "bf16": BF16, "f32": F32}[COMPUTE]
    AF = mybir.ActivationFunctionType
    MULT = mybir.AluOpType.mult
    ADD = mybir.AluOpType.add

    nc = bacc.Bacc(None, target_bir_lowering=False, num_devices=8)

    xT = nc.declare_dram_parameter("xT", [D, L], SB, isOutput=False)
    cosT = nc.declare_dram_parameter("cosT", [Dh, L], SB, isOutput=False)
    sinT = nc.declare_dram_parameter("sinT", [Dh, L], SB, isOutput=False)
    wq = nc.declare_dram_parameter("wq", [D, HD], SB, isOutput=False)
    wk = nc.declare_dram_parameter("wk", [D, Dh], SB, isOutput=False)
    wv = nc.declare_dram_parameter("wv", [D, Dh], SB, isOutput=False)
    wo = nc.declare_dram_parameter("wo", [D, LB], SB, isOutput=False)
    rot = nc.declare_dram_parameter("rot", [Dh, Dh], SB, isOutput=False)
    iden = nc.declare_dram_parameter("iden", [P, P], F32, isOutput=False)
    ones = nc.declare_dram_parameter("ones", [P, P], SB, isOutput=False)
    masks = nc.declare_dram_parameter("masks", [P, P], SB, isOutput=False)
    outT = nc.declare_dram_parameter("outT", [LB, L], F32, isOutput=True)

    def mmul(out, lhsT, rhs, **kw):
        nc.tensor.matmul(out, lhsT, rhs, **kw)

    with tile.TileContext(nc) as tc, \
         nc.allow_low_precision(reason="softmax in bf16, approx reciprocal"):
        with tc.tile_pool(name="dram", bufs=1, space="DRAM") as dram, \
             tc.tile_pool(name="const", bufs=1) as cpool, \
             tc.tile_pool(name="acts", bufs=1) as apool, \
             tc.tile_pool(name="wpool", bufs=1) as wpool, \
             tc.tile_pool(name="xpool", bufs=1) as xpool, \
             tc.tile_pool(name="work", bufs=1) as tpool, \
             tc.tile_pool(name="psum", bufs=1, space="PSUM") as psum:

            ag_in = [dram.tile([NG * P, LB], SB, name=f"agin{j}")
                     for j in range(NLB)]
            ag_out = [dram.tile([NG * NG * P, LB], SB, name=f"agout{j}")
                      for j in range(NLB)]
            ag_wu_in = dram.tile([NG * P, 64], SB, name="agwui")
            ag_wu_out = dram.tile([NG * NG * P, 64], SB, name="agwuo")

            # ---------------- constants ----------------
            rot_t = cpool.tile([Dh, Dh], SB)
            iden_t = cpool.tile([P, P], F32)
            ones_t = cpool.tile([P, P], SB)
            mask_t = cpool.tile([P, P], SB)
            cos_t = cpool.tile([Dh, L], SB)
            sin_t = cpool.tile([Dh, L], SB)
            nc.sync.dma_start(rot_t[:], rot[:])
            nc.sync.dma_start(iden_t[:], iden[:])
            nc.sync.dma_start(ones_t[:], ones[:])
            nc.sync.dma_start(mask_t[:], masks[:])
            nc.sync.dma_start(cos_t[:], cosT[:])
            nc.sync.dma_start(sin_t[:], sinT[:])

            # tiny warm-up AllGather: absorbs first-collective startup cost
            # (DMA ring init etc, ~25us) during the projection phase
            nc.sync.dma_start(ag_wu_in[:, :], xT[0:NG * P, 0:64])
            nc.gpsimd.collective_compute(
                "AllGather",
                mybir.AluOpType.bypass,
                replica_groups=[[0, 1, 2, 3], [4, 5, 6, 7]],
                ins=[ag_wu_in[:, :].opt()],
                outs=[ag_wu_out[:, :].opt()],
            )

            # persistent activations
            qkT = apool.tile([Dh, HPC + 1, L], SB)  # 4 q heads + k
            vnat = apool.tile([P, NLT, Dh], SB)     # v natural layout

            # weights (DMA'd up front; wo deferred)
            wq_t = wpool.tile([P, NKT, HD], SB)
            wk_t = wpool.tile([P, NKT, Dh], SB)
            wv_t = wpool.tile([P, NKT, Dh], SB)
            wo_t = wpool.tile([P, NKT, LB], SB)

            xt = {}

            def xt_load(blk, ck=4):
                lqs = slice(blk * LB, (blk + 1) * LB)
                t = xpool.tile([P, NKT, LB], SB, name=f"xt{blk}",
                               tag="xt3", bufs=4)
                for k0 in range(0, NKT, ck):
                    nc.sync.dma_start(
                        t[:, k0:k0 + ck, :],
                        xT[k0 * P:(k0 + ck) * P, lqs].rearrange(
                            "(j p) n -> p j n", p=P))
                xt[blk] = t

            gt = {}

            def gt_load(blk):
                t = xpool.tile([P, NKT, LB], SB, name=f"gt{blk}",
                               tag="gt3", bufs=2)
                for j in range(NKT):
                    nc.gpsimd.dma_start(
                        t[:, j, :], ag_out[blk][j * P:(j + 1) * P, :])
                gt[blk] = t

            # x block 0 + wq first (fine chunks, parallel queues), then wk/wv
            xt_load(0, ck=2)
            for k0 in range(0, NKT, 2):
                nc.sync.dma_start(
                    wq_t[:, k0:k0 + 2, :],
                    wq[k0 * P:(k0 + 2) * P, :].rearrange(
                        "(j p) n -> p j n", p=P))
            for k0 in range(0, NKT, 4):
                nc.sync.dma_start(
                    wk_t[:, k0:k0 + 4, :],
                    wk[k0 * P:(k0 + 4) * P, :].rearrange(
                        "(j p) n -> p j n", p=P))
                nc.sync.dma_start(
                    wv_t[:, k0:k0 + 4, :],
                    wv[k0 * P:(k0 + 4) * P, :].rearrange(
                        "(j p) n -> p j n", p=P))
            for b in range(1, NLB):
                xt_load(b)

            def wo_load():
                for k0 in range(0, NKT, 4):
                    nc.sync.dma_start(
                        wo_t[:, k0:k0 + 4, :],
                        wo[k0 * P:(k0 + 4) * P, :].rearrange(
                            "(j p) n -> p j n", p=P))

            # ---------------- phases ----------------
            def proj(blk):
                lqs = slice(blk * LB, (blk + 1) * LB)
                x3 = xt[blk]
                vts = xpool.tile([Dh, LB], F32, name=f"vts{blk}",
                                 tag="vts", bufs=2)
                for m in range(HPC + 2):
                    pj = psum.tile([P, LB], F32, name=f"pj{blk}_{m}",
                                   tag="pj", bufs=2)
                    for kk in range(NKT):
                        if m < HPC:
                            wsl = wq_t[:, kk, m * Dh:(m + 1) * Dh]
                        elif m == HPC:
                            wsl = wk_t[:, kk, :]
                        else:
                            wsl = wv_t[:, kk, :]
                        mmul(pj[:], wsl, x3[:, kk, :],
                             start=(kk == 0), stop=(kk == NKT - 1))
                    if m == HPC + 1:
                        # v: stage vT, transpose to natural layout
                        nc.vector.tensor_copy(vts[:], pj[:])
                        for j in range(4):
                            vp = psum.tile([P, LB], F32, name=f"vp{blk}_{j}",
                                           tag="sq", bufs=2)
                            nc.tensor.transpose(
                                vp[:, :P], vts[:, j * P:(j + 1) * P],
                                iden_t[:])
                            nc.vector.tensor_copy(
                                vnat[:, blk * 4 + j, :], vp[:, :P])
                    else:
                        # RoPE: q' = cos*q + sin*(R q)
                        qs = tpool.tile([Dh, LB], SB, name=f"qs{blk}_{m}",
                                        tag="qs", bufs=2)
                        nc.vector.tensor_copy(qs[:], pj[:])
                        rq = psum.tile([Dh, LB], F32, name=f"rq{blk}_{m}",
                                       tag="sq", bufs=2)
                        mmul(rq[:], rot_t[:], qs[:], start=True, stop=True)
                        t1 = tpool.tile([Dh, LB], SB, name=f"t1_{blk}_{m}",
                                        tag="t1", bufs=2)
                        nc.vector.tensor_tensor(
                            out=t1[:], in0=qs[:], in1=cos_t[:, lqs], op=MULT)
                        t2 = tpool.tile([Dh, LB], SB, name=f"t2_{blk}_{m}",
                                        tag="t2", bufs=2)
                        nc.vector.tensor_tensor(
                            out=t2[:], in0=rq[:], in1=sin_t[:, lqs], op=MULT)
                        nc.vector.tensor_tensor(
                            out=qkT[:, m, lqs], in0=t1[:], in1=t2[:], op=ADD)

            def attn(blk):
                nlk = (blk + 1) * NG  # causal lk tiles
                for h in range(HPC):
                    ot = psum.tile([Dh, LB], F32, name=f"ot{blk}_{h}",
                                   tag="ot", bufs=2)
                    racc = tpool.tile([P, LB], SB, name=f"racc{blk}_{h}",
                                      tag="racc", bufs=2)
                    for i in range(nlk):
                        di = i - NG * blk
                        c0 = di * P if di > 0 else 0
                        st = psum.tile([P, LB], F32, name=f"st{blk}_{h}_{i}",
                                       tag="sq", bufs=2)
                        mmul(st[:, c0:],
                             qkT[:, HPC, i * P:(i + 1) * P],
                             qkT[:, h, blk * LB + c0:(blk + 1) * LB],
                             start=True, stop=True)
                        if di >= 0:
                            nc.vector.tensor_tensor(
                                out=st[:, c0:c0 + P], in0=st[:, c0:c0 + P],
                                in1=mask_t[:], op=ADD)
                        pt = tpool.tile([P, LB], SB, name=f"pt{blk}_{h}_{i}",
                                        tag="pt", bufs=3)
                        nc.scalar.activation(pt[:, c0:], st[:, c0:],
                                             AF.Exp, scale=SCALE)
                        if i == 0:
                            mmul(ot[:], vnat[:, i, :], pt[:],
                                 start=True, stop=(i == nlk - 1))
                            nc.vector.tensor_copy(racc[:], pt[:])
                        else:
                            mmul(ot[:, c0:], vnat[:, i, :], pt[:, c0:],
                                 start=False, stop=(i == nlk - 1))
                            nc.vector.tensor_tensor(
                                out=racc[:, c0:], in0=racc[:, c0:],
                                in1=pt[:, c0:], op=ADD)
                    # denominator: ones-matmul broadcasts column sums to all
                    # 128 partitions; approx reciprocal is 1 DVE instruction
                    rsb = psum.tile([P, LB], F32, name=f"rsb{blk}_{h}",
                                    tag="sq", bufs=2)
                    mmul(rsb[:], ones_t[:], racc[:], start=True, stop=True)
                    rb = tpool.tile([P, LB], F32, name=f"rb{blk}_{h}",
                                    tag="rb", bufs=2)
                    nc.vector.reciprocal_approx_fast(rb[:], rsb[:])
                    od = tpool.tile([Dh, LB], SB, name=f"od{blk}_{h}",
                                    tag="od", bufs=2)
                    nc.vector.tensor_tensor(
                        out=od[:], in0=ot[:], in1=rb[:], op=MULT)
                    nc.scalar.dma_start(
                        ag_in[blk][h * P:(h + 1) * P, :], od[:])
                nc.gpsimd.collective_compute(
                    "AllGather",
                    mybir.AluOpType.bypass,
                    replica_groups=[[0, 1, 2, 3], [4, 5, 6, 7]],
                    ins=[ag_in[blk][:, :].opt()],
                    outs=[ag_out[blk][:, :].opt()],
                )

            def outproj(blk):
                lqs = slice(blk * LB, (blk + 1) * LB)
                g3 = gt[blk]
                for nt in range(4):
                    fp = psum.tile([P, LB], F32, name=f"fp{blk}_{nt}",
                                   tag="fp", bufs=2)
                    for kk in range(NKT):
                        mmul(fp[:], wo_t[:, kk, nt * P:(nt + 1) * P],
                             g3[:, kk, :],
                             start=(kk == 0), stop=(kk == NKT - 1))
                    ft = tpool.tile([P, LB], F32, name=f"ft{blk}_{nt}",
                                    tag="ft", bufs=2)
                    nc.vector.tensor_copy(ft[:], fp[:])
                    nc.scalar.dma_start(outT[nt * P:(nt + 1) * P, lqs],
                                        ft[:])

            # ---------------- pipeline ----------------
            for blk in range(NLB):
                proj(blk)
                attn(blk)
                if blk >= 1:
                    gt_load(blk - 1)
                if blk == 0:
                    wo_load()
            gt_load(NLB - 1)
            for blk in range(NLB):
                outproj(blk)
    return nc


def _np_dt():
    if COMPUTE == "bf16":
        import ml_dtypes
        return ml_dtypes.bfloat16
    return np.float32


def _prep(hidden_states, cos, sin, Wq, Wk, Wv, Wo):
    rot = np.zeros((Dh, Dh), dtype=np.float32)
    for p in range(Dh // 2):
        rot[p, p + Dh // 2] = 1.0
        rot[p + Dh // 2, p] = -1.0
    iden = np.eye(P, dtype=np.float32)
    ones = np.ones((P, P), dtype=np.float32)
    # triangular tile mask: masked where kl > qq (S^T diagonal tile)
    kl = np.arange(P)[:, None]
    qq = np.arange(P)[None, :]
    masks = np.where(kl > qq, NEG, 0.0).astype(np.float32)

    cosT = np.ascontiguousarray(cos.T)
    sinT = np.ascontiguousarray(sin.T)
    ndt = _np_dt()
    maps = []
    for c in range(8):
        b, g = divmod(c, NG)
        maps.append({
            "xT": np.ascontiguousarray(hidden_states[b].T).astype(ndt),
            "cosT": cosT.astype(ndt), "sinT": sinT.astype(ndt),
            "wq": np.ascontiguousarray(Wq[:, g * HD:(g + 1) * HD]).astype(ndt),
            "wk": np.ascontiguousarray(Wk[:, g * Dh:(g + 1) * Dh]).astype(ndt),
            "wv": np.ascontiguousarray(Wv[:, g * Dh:(g + 1) * Dh]).astype(ndt),
            "wo": np.ascontiguousarray(Wo[:, g * LB:(g + 1) * LB]).astype(ndt),
            "rot": rot.astype(ndt), "iden": iden,
            "ones": ones.astype(ndt), "masks": masks.astype(ndt),
        })
    return maps


def kernel(hidden_states, cos, sin, Wq, Wk, Wv, Wo):
    from concourse.bass_utils import run_bass_kernel_spmd

    hidden_states = np.asarray(hidden_states, dtype=np.float32)
    cos = np.asarray(cos, dtype=np.float32)
    sin = np.asarray(sin, dtype=np.float32)
    Wq = np.asarray(Wq, dtype=np.float32)
    Wk = np.asarray(Wk, dtype=np.float32)
    Wv = np.asarray(Wv, dtype=np.float32)
    Wo = np.asarray(Wo, dtype=np.float32)

    if "nc" not in _cache:
        nc = _build()
        nc.finalize()
        _cache["nc"] = nc
    nc = _cache["nc"]
    in_maps = _prep(hidden_states, cos, sin, Wq, Wk, Wv, Wo)
    res = run_bass_kernel_spmd(nc, in_maps, list(range(8)))
    _cache["last_result"] = res
    out = np.empty((B, L, D), dtype=np.float32)
    for c in range(8):
        b, g = divmod(c, NG)
        out[b, :, g * LB:(g + 1) * LB] = res.results[c]["outT"].T
    return out


# revision 9
# speedup vs baseline: 1.2313x; 1.1143x over previous
"""GQA attention (B=2, L=2048, D=2048, H=16, KVH=4, Dh=128, RoPE, causal)
distributed over 8 TRN2 NeuronCores.

Sharding: DP on batch (2) x TP on head-groups (4). Core c handles batch
b=c//4, head group g=c%4 (q heads 4g..4g+3, kv head g). Per lq-block the
attention outputs O_g^T (4 heads, [512, LB], transposed) are AllGather'd
in ONE collective across the 4 TP cores; each core then computes a
distinct 512-column slice of the output projection (Wo column-sharded),
so the big [L, D] activation never rides a collective.

v2 restructure vs v1:
- per-block software pipeline proj(b) -> attn(b) -> AG(b) -> outproj(b)
  so the first AllGather issues at ~25us (was 172us) and comm latency
  hides under later blocks' projection/attention compute.
- 4 batched AllGathers (one per block) instead of 16 per-(block,head)
  calls: same bytes, 1/4 the per-call barrier latency.
- softmax denominator: all-ones [128,128] matmul broadcasts column sums
  to every partition in one PE op; 1/x via the single-instruction DVE
  reciprocal_approx_fast on [128,512] (v1: [1,512] bit-exact reciprocal
  at 3.3us + 1us GpSimd PartitionBroadcast, both on the critical path).
- diagonal S/PV tiles compute only the live [c0:] column range (no
  memsets, narrower matmuls).
- DMA order: x block 0 first, then qkv weights k-ascending, wo last, so
  the first projection matmul starts ~8us into the kernel.
"""
import sys
sys.path.insert(0, '/opt/trn_rl_repo')
import math
import numpy as np

B, L, D = 2, 2048, 2048
H, KVH, Dh = 16, 4, 128
HPC = H // KVH          # q heads per core = 4
NG = 4                  # TP group size
P = 128                 # partitions
LB = 512                # lq block size
NLB = L // LB           # 4 lq blocks
NKT = D // P            # 16 k-tiles over D
NLT = L // P            # 16 tiles over L
HD = HPC * Dh           # 512
SCALE = 1.0 / math.sqrt(Dh)
NEG = -1.0e30

_cache = {}
COMPUTE = "bf16"  # "f32r" | "bf16" | "f32"


def _build():
    from concourse import bacc, bass, mybir, tile

    F32 = mybir.dt.float32
    F32R = mybir.dt.float32r
    BF16 = mybir.dt.bfloat16
    SB = {"f32r": F32R, "bf16": BF16, "f32": F32}[COMPUTE]
    AF = mybir.ActivationFunctionType
    MULT = mybir.AluOpType.mult
    ADD = mybir.AluOpType.add

    nc = bacc.Bacc(None, target_bir_lowering=False, num_devices=8)

    xT = nc.declare_dram_parameter("xT", [D, L], SB, isOutput=False)
    cosT = nc.declare_dram_parameter("cosT", [Dh, L], SB, isOutput=False)
    sinT = nc.declare_dram_parameter("sinT", [Dh, L], SB, isOutput=False)
    wq = nc.declare_dram_parameter("wq", [D, HD], SB, isOutput=False)
    wk = nc.declare_dram_parameter("wk", [D, Dh], SB, isOutput=False)
    wv = nc.declare_dram_parameter("wv", [D, Dh], SB, isOutput=False)
    wo = nc.declare_dram_parameter("wo", [D, LB], SB, isOutput=False)
    rot = nc.declare_dram_parameter("rot", [Dh, Dh], SB, isOutput=False)
    iden = nc.declare_dram_parameter("iden", [P, P], F32, isOutput=False)
    ones = nc.declare_dram_parameter("ones", [P, P], SB, isOutput=False)
    masks = nc.declare_dram_parameter("masks", [P, P], SB, isOutput=False)
    outT = nc.declare_dram_parameter("outT", [LB, L], F32, isOutput=True)

    def mmul(out, lhsT, rhs, **kw):
        nc.tensor.matmul(out, lhsT, rhs, **kw)

    with tile.TileContext(nc) as tc, \
         nc.allow_low_precision(reason="softmax in bf16, approx reciprocal"):
        with tc.tile_pool(name="dram", bufs=1, space="DRAM") as dram, \
             tc.tile_pool(name="const", bufs=1) as cpool, \
             tc.tile_pool(name="acts", bufs=1) as apool, \
             tc.tile_pool(name="wpool", bufs=1) as wpool, \
             tc.tile_pool(name="xpool", bufs=1) as xpool, \
             tc.tile_pool(name="work", bufs=1) as tpool, \
             tc.tile_pool(name="psum", bufs=1, space="PSUM") as psum:

            ag_in = [dram.tile([NG * P, LB], SB, name=f"agin{j}")
                     for j in range(NLB)]
            ag_out = [dram.tile([NG * NG * P, LB], SB, name=f"agout{j}")
                      for j in range(NLB)]
            ag_wu_in = dram.tile([NG * P, 64], SB, name="agwui")
            ag_wu_out = dram.tile([NG * NG * P, 64], SB, name="agwuo")
            ag3_out = [dram.tile([NG * 2 * P, LB], SB, name=f"agout3{x}")
                       for x in "ab"]

            # ---------------- constants ----------------
            rot_t = cpool.tile([Dh, Dh], SB)
            iden_t = cpool.tile([P, P], F32)
            ones_t = cpool.tile([P, P], SB)
            mask_t = cpool.tile([P, P], SB)
            cos_t = cpool.tile([Dh, L], SB)
            sin_t = cpool.tile([Dh, L], SB)
            nc.sync.dma_start(rot_t[:], rot[:])
            nc.sync.dma_start(iden_t[:], iden[:])
            nc.sync.dma_start(ones_t[:], ones[:])
            nc.sync.dma_start(mask_t[:], masks[:])
            nc.sync.dma_start(cos_t[:], cosT[:])
            nc.sync.dma_start(sin_t[:], sinT[:])

            # tiny warm-up AllGather: absorbs first-collective startup cost
            # (DMA ring init etc, ~25us) during the projection phase
            nc.sync.dma_start(ag_wu_in[:, :], xT[0:NG * P, 0:64])
            nc.gpsimd.collective_compute(
                "AllGather",
                mybir.AluOpType.bypass,
                replica_groups=[[0, 1, 2, 3], [4, 5, 6, 7]],
                ins=[ag_wu_in[:, :].opt()],
                outs=[ag_wu_out[:, :].opt()],
            )

            # persistent activations
            qkT = apool.tile([Dh, HPC + 1, L], SB)  # 4 q heads + k
            vnat = apool.tile([P, NLT, Dh], SB)     # v natural layout

            # weights (DMA'd up front; wo deferred)
            wq_t = wpool.tile([P, NKT, HD], SB)
            wk_t = wpool.tile([P, NKT, Dh], SB)
            wv_t = wpool.tile([P, NKT, Dh], SB)
            wo_t = wpool.tile([P, NKT, LB], SB)

            xt = {}

            def xt_load(blk, ck=4):
                lqs = slice(blk * LB, (blk + 1) * LB)
                t = xpool.tile([P, NKT, LB], SB, name=f"xt{blk}",
                               tag="xt3", bufs=4)
                for k0 in range(0, NKT, ck):
                    nc.sync.dma_start(
                        t[:, k0:k0 + ck, :],
                        xT[k0 * P:(k0 + ck) * P, lqs].rearrange(
                            "(j p) n -> p j n", p=P))
                xt[blk] = t

            gt = {}

            def gt_load(blk):
                t = xpool.tile([P, NKT, LB], SB, name=f"gt{blk}",
                               tag="gt3", bufs=2)
                if blk == NLB - 1:
                    # slots j=HPC*g+h; half a carries h in {0,1}, b {2,3}
                    for half in range(2):
                        for g in range(NG):
                            for hh in range(2):
                                j = HPC * g + half * 2 + hh
                                nc.gpsimd.dma_start(
                                    t[:, j, :],
                                    ag3_out[half][g * 2 * P + hh * P:
                                                  g * 2 * P + (hh + 1) * P, :])
                else:
                    for j in range(NKT):
                        nc.gpsimd.dma_start(
                            t[:, j, :], ag_out[blk][j * P:(j + 1) * P, :])
                gt[blk] = t

            # x block 0 + wq first (fine chunks, parallel queues), then wk/wv
            xt_load(0, ck=2)
            for k0 in range(0, NKT, 2):
                nc.sync.dma_start(
                    wq_t[:, k0:k0 + 2, :],
                    wq[k0 * P:(k0 + 2) * P, :].rearrange(
                        "(j p) n -> p j n", p=P))
            for k0 in range(0, NKT, 4):
                nc.sync.dma_start(
                    wk_t[:, k0:k0 + 4, :],
                    wk[k0 * P:(k0 + 4) * P, :].rearrange(
                        "(j p) n -> p j n", p=P))
                nc.sync.dma_start(
                    wv_t[:, k0:k0 + 4, :],
                    wv[k0 * P:(k0 + 4) * P, :].rearrange(
                        "(j p) n -> p j n", p=P))
            for b in range(1, NLB):
                xt_load(b)

            def wo_load():
                for k0 in range(0, NKT, 4):
                    nc.sync.dma_start(
                        wo_t[:, k0:k0 + 4, :],
                        wo[k0 * P:(k0 + 4) * P, :].rearrange(
                            "(j p) n -> p j n", p=P))

            # ---------------- phases ----------------
            def proj(blk):
                lqs = slice(blk * LB, (blk + 1) * LB)
                x3 = xt[blk]
                vts = xpool.tile([Dh, LB], F32, name=f"vts{blk}",
                                 tag="vts", bufs=2)
                for m in range(HPC + 2):
                    pj = psum.tile([P, LB], F32, name=f"pj{blk}_{m}",
                                   tag="pj", bufs=2)
                    for kk in range(NKT):
                        if m < HPC:
                            wsl = wq_t[:, kk, m * Dh:(m + 1) * Dh]
                        elif m == HPC:
                            wsl = wk_t[:, kk, :]
                        else:
                            wsl = wv_t[:, kk, :]
                        mmul(pj[:], wsl, x3[:, kk, :],
                             start=(kk == 0), stop=(kk == NKT - 1))
                    if m == HPC + 1:
                        # v: stage vT, transpose to natural layout
                        nc.vector.tensor_copy(vts[:], pj[:])
                        for j in range(4):
                            vp = psum.tile([P, LB], F32, name=f"vp{blk}_{j}",
                                           tag="sq", bufs=4)
                            nc.tensor.transpose(
                                vp[:, :P], vts[:, j * P:(j + 1) * P],
                                iden_t[:])
                            nc.vector.tensor_copy(
                                vnat[:, blk * 4 + j, :], vp[:, :P])
                    else:
                        # RoPE: q' = cos*q + sin*(R q)
                        qs = tpool.tile([Dh, LB], SB, name=f"qs{blk}_{m}",
                                        tag="qs", bufs=2)
                        nc.vector.tensor_copy(qs[:], pj[:])
                        rq = psum.tile([Dh, LB], F32, name=f"rq{blk}_{m}",
                                       tag="sq", bufs=4)
                        mmul(rq[:], rot_t[:], qs[:], start=True, stop=True)
                        t1 = tpool.tile([Dh, LB], SB, name=f"t1_{blk}_{m}",
                                        tag="t1", bufs=2)
                        nc.vector.tensor_tensor(
                            out=t1[:], in0=qs[:], in1=cos_t[:, lqs], op=MULT)
                        t2 = tpool.tile([Dh, LB], SB, name=f"t2_{blk}_{m}",
                                        tag="t2", bufs=2)
                        nc.vector.tensor_tensor(
                            out=t2[:], in0=rq[:], in1=sin_t[:, lqs], op=MULT)
                        nc.vector.tensor_tensor(
                            out=qkT[:, m, lqs], in0=t1[:], in1=t2[:], op=ADD)

            def attn(blk):
                nlk = (blk + 1) * NG  # causal lk tiles
                for h in range(HPC):
                    ot = psum.tile([Dh, LB], F32, name=f"ot{blk}_{h}",
                                   tag="ot", bufs=2)
                    racc = tpool.tile([P, LB], SB, name=f"racc{blk}_{h}",
                                      tag="racc", bufs=2)
                    for i in range(nlk):
                        di = i - NG * blk
                        c0 = di * P if di > 0 else 0
                        st = psum.tile([P, LB], F32, name=f"st{blk}_{h}_{i}",
                                       tag="sq", bufs=4)
                        mmul(st[:, c0:],
                             qkT[:, HPC, i * P:(i + 1) * P],
                             qkT[:, h, blk * LB + c0:(blk + 1) * LB],
                             start=True, stop=True)
                        if di >= 0:
                            nc.vector.tensor_tensor(
                                out=st[:, c0:c0 + P], in0=st[:, c0:c0 + P],
                                in1=mask_t[:], op=ADD)
                        pt = tpool.tile([P, LB], SB, name=f"pt{blk}_{h}_{i}",
                                        tag="pt", bufs=4)
                        nc.scalar.activation(pt[:, c0:], st[:, c0:],
                                             AF.Exp, scale=SCALE)
                        if i == 0:
                            mmul(ot[:], vnat[:, i, :], pt[:],
                                 start=True, stop=(i == nlk - 1))
                            nc.vector.tensor_copy(racc[:], pt[:])
                        else:
                            mmul(ot[:, c0:], vnat[:, i, :], pt[:, c0:],
                                 start=False, stop=(i == nlk - 1))
                            nc.vector.tensor_tensor(
                                out=racc[:, c0:], in0=racc[:, c0:],
                                in1=pt[:, c0:], op=ADD)
                    # denominator: ones-matmul broadcasts column sums to all
                    # 128 partitions; approx reciprocal is 1 DVE instruction
                    rsb = psum.tile([P, LB], F32, name=f"rsb{blk}_{h}",
                                    tag="sq", bufs=4)
                    mmul(rsb[:], ones_t[:], racc[:], start=True, stop=True)
                    rb = tpool.tile([P, LB], F32, name=f"rb{blk}_{h}",
                                    tag="rb", bufs=2)
                    nc.vector.reciprocal_approx_fast(rb[:], rsb[:])
                    od = tpool.tile([Dh, LB], SB, name=f"od{blk}_{h}",
                                    tag="od", bufs=2)
                    nc.vector.tensor_tensor(
                        out=od[:], in0=ot[:], in1=rb[:], op=MULT)
                    nc.scalar.dma_start(
                        ag_in[blk][h * P:(h + 1) * P, :], od[:])
                    if blk == NLB - 1 and h == 1:
                        # first half of last block's AG fires early so the
                        # tail collective overlaps the rest of attention
                        nc.gpsimd.collective_compute(
                            "AllGather",
                            mybir.AluOpType.bypass,
                            replica_groups=[[0, 1, 2, 3], [4, 5, 6, 7]],
                            ins=[ag_in[blk][0:2 * P, :].opt()],
                            outs=[ag3_out[0][:, :].opt()],
                        )
                if blk == NLB - 1:
                    nc.gpsimd.collective_compute(
                        "AllGather",
                        mybir.AluOpType.bypass,
                        replica_groups=[[0, 1, 2, 3], [4, 5, 6, 7]],
                        ins=[ag_in[blk][2 * P:4 * P, :].opt()],
                        outs=[ag3_out[1][:, :].opt()],
                    )
                else:
                    nc.gpsimd.collective_compute(
                        "AllGather",
                        mybir.AluOpType.bypass,
                        replica_groups=[[0, 1, 2, 3], [4, 5, 6, 7]],
                        ins=[ag_in[blk][:, :].opt()],
                        outs=[ag_out[blk][:, :].opt()],
                    )

            def outproj(blk):
                lqs = slice(blk * LB, (blk + 1) * LB)
                g3 = gt[blk]
                if blk == NLB - 1:
                    order = [HPC * g + half * 2 + hh for half in range(2)
                             for g in range(NG) for hh in range(2)]
                else:
                    order = list(range(NKT))
                for nt in range(4):
                    fp = psum.tile([P, LB], F32, name=f"fp{blk}_{nt}",
                                   tag="pj", bufs=2)
                    for j, kk in enumerate(order):
                        mmul(fp[:], wo_t[:, kk, nt * P:(nt + 1) * P],
                             g3[:, kk, :],
                             start=(j == 0), stop=(j == NKT - 1))
                    ft = tpool.tile([P, LB], F32, name=f"ft{blk}_{nt}",
                                    tag="ft", bufs=2)
                    nc.vector.tensor_copy(ft[:], fp[:])
                    nc.scalar.dma_start(outT[nt * P:(nt + 1) * P, lqs],
                                        ft[:])

            # ---------------- pipeline ----------------
            for blk in range(NLB):
                proj(blk)
                if blk >= 1:
                    gt_load(blk - 1)
                attn(blk)
                if blk == 0:
                    wo_load()
            gt_load(NLB - 1)
            for blk in range(NLB):
                outproj(blk)
    return nc


def _np_dt():
    if COMPUTE == "bf16":
        import ml_dtypes
        return ml_dtypes.bfloat16
    return np.float32


def _prep(hidden_states, cos, sin, Wq, Wk, Wv, Wo):
    rot = np.zeros((Dh, Dh), dtype=np.float32)
    for p in range(Dh // 2):
        rot[p, p + Dh // 2] = 1.0
        rot[p + Dh // 2, p] = -1.0
    iden = np.eye(P, dtype=np.float32)
    ones = np.ones((P, P), dtype=np.float32)
    # triangular tile mask: masked where kl > qq (S^T diagonal tile)
    kl = np.arange(P)[:, None]
    qq = np.arange(P)[None, :]
    masks = np.where(kl > qq, NEG, 0.0).astype(np.float32)

    cosT = np.ascontiguousarray(cos.T)
    sinT = np.ascontiguousarray(sin.T)
    ndt = _np_dt()
    maps = []
    for c in range(8):
        b, g = divmod(c, NG)
        maps.append({
            "xT": np.ascontiguousarray(hidden_states[b].T).astype(ndt),
            "cosT": cosT.astype(ndt), "sinT": sinT.astype(ndt),
            "wq": np.ascontiguousarray(Wq[:, g * HD:(g + 1) * HD]).astype(ndt),
            "wk": np.ascontiguousarray(Wk[:, g * Dh:(g + 1) * Dh]).astype(ndt),
            "wv": np.ascontiguousarray(Wv[:, g * Dh:(g + 1) * Dh]).astype(ndt),
            "wo": np.ascontiguousarray(Wo[:, g * LB:(g + 1) * LB]).astype(ndt),
            "rot": rot.astype(ndt), "iden": iden,
            "ones": ones.astype(ndt), "masks": masks.astype(ndt),
        })
    return maps


def kernel(hidden_states, cos, sin, Wq, Wk, Wv, Wo):
    from concourse.bass_utils import run_bass_kernel_spmd

    hidden_states = np.asarray(hidden_states, dtype=np.float32)
    cos = np.asarray(cos, dtype=np.float32)
    sin = np.asarray(sin, dtype=np.float32)
    Wq = np.asarray(Wq, dtype=np.float32)
    Wk = np.asarray(Wk, dtype=np.float32)
    Wv = np.asarray(Wv, dtype=np.float32)
    Wo = np.asarray(Wo, dtype=np.float32)

    if "nc" not in _cache:
        nc = _build()
        nc.finalize()
        _cache["nc"] = nc
    nc = _cache["nc"]
    in_maps = _prep(hidden_states, cos, sin, Wq, Wk, Wv, Wo)
    res = run_bass_kernel_spmd(nc, in_maps, list(range(8)))
    _cache["last_result"] = res
    out = np.empty((B, L, D), dtype=np.float32)
    for c in range(8):
        b, g = divmod(c, NG)
        out[b, :, g * LB:(g + 1) * LB] = res.results[c]["outT"].T
    return out
